# revision 8
# baseline (speedup 1.0000x reference)
"""RBF kernel matrix on 8 TRN2 NeuronCores.

out[i, j] = exp(-(||x_i||^2 + ||y_j||^2 - 2 x_i.y_j))

Sharding: x row-wise across 8 cores (1024 rows each), y replicated.
Each core computes a (1024, 8192) tile of the output.

Per-core algorithm:
  exp(-d2) = Exp(2 * (xy - 0.5*y2_j) + (-x2_i))
  - xy via bf16 matmuls (2 K-tiles of 128) accumulated in PSUM
  - -0.5*y2_j folded in as a K=1 matmul with a constant ones lhsT row
  - -x2_i applied as the per-partition bias of the ScalarE Exp activation
    (scale=2.0 applied by the same instruction)
bf16 operand transposes (contraction dim must be on partitions) are done
with the DMA xbar transpose from a bf16 DRAM staging copy.
"""

import numpy as np

import concourse.bass as bass
import concourse.bacc as bacc
import concourse.mybir as mybir
from concourse import tile
from concourse.bass_utils import run_bass_kernel_spmd

N, M, D = 8192, 8192, 256
NCORES = 8
NSH = N // NCORES  # 1024 rows of x per core

F32 = mybir.dt.float32
BF16 = mybir.dt.bfloat16
AF = mybir.ActivationFunctionType
AX = mybir.AxisListType

_NC_CACHE = {}


def _build_nc() -> bass.Bass:
    # Bacc (not plain Bass): its compile() runs generate_event_semaphores,
    # which splits multi-wait instructions to satisfy TRN2's 1-wait limit.
    nc = bacc.Bacc("TRN2", target_bir_lowering=False, debug=False)
    x = nc.dram_tensor("x", (NSH, D), F32, kind="ExternalInput")
    y = nc.dram_tensor("y", (M, D), F32, kind="ExternalInput")
    out = nc.dram_tensor("out", (NSH, M), F32, kind="ExternalOutput")

    XB = NSH // 128  # 8 i-blocks per core
    YB = M // 128  # 64 row-tiles of y

    with tile.TileContext(nc) as tc:
        with (
            tc.tile_pool(name="dram", bufs=1, space="DRAM") as dpool,
            tc.tile_pool(name="const", bufs=1) as cpool,
            tc.tile_pool(name="persist", bufs=1) as ppool,
            tc.tile_pool(name="stage", bufs=3) as spool,
            tc.tile_pool(name="outp", bufs=3) as opool,
            tc.tile_pool(name="psum", bufs=2, space="PSUM") as pspool,
        ):
            # DRAM staging for bf16 copies (xbar transpose needs 2-byte dtype)
            y_bf = dpool.tile((M, D), BF16)
            x_bf = dpool.tile((NSH, D), BF16)

            # Persistent SBUF tensors
            yT0 = ppool.tile((128, M), BF16)  # y^T, d in [0,128)
            yT1 = ppool.tile((128, M), BF16)  # y^T, d in [128,256)
            xT0 = ppool.tile((128, NSH), BF16)
            xT1 = ppool.tile((128, NSH), BF16)
            y2row = ppool.tile((1, M), BF16)  # holds -0.5 * ||y_j||^2
            negx2 = ppool.tile((128, XB), F32)  # col b = -||x_i||^2 for i-block b

            ones_row = cpool.tile((1, 128), BF16)
            nc.vector.memset(ones_row[:, :], 1.0)
            neghalf_col = cpool.tile((128, 1), BF16)
            nc.vector.memset(neghalf_col[:, :], -0.5)

            # ---- x: load f32 once, x2 stats, bf16 staging, transpose ----
            x_re = x[:, :].rearrange("(t p) d -> p t d", p=128)
            xf = spool.tile((128, XB * D), F32, bufs=1)
            nc.sync.dma_start(xf[:, :], x_re)
            xsq = spool.tile((128, XB * D), F32, bufs=1)
            nc.vector.tensor_mul(xsq[:, :], xf[:, :], xf[:, :])
            x2tmp = spool.tile((128, XB), F32, bufs=1)
            for b in range(XB):
                nc.vector.reduce_sum(
                    x2tmp[:, b : b + 1], xsq[:, b * D : (b + 1) * D], axis=AX.X
                )
            nc.vector.tensor_scalar_mul(negx2[:, :], x2tmp[:, :], -1.0)

            xb16 = spool.tile((128, XB * D), BF16, bufs=1)
            nc.vector.tensor_copy(xb16[:, :], xf[:, :])
            nc.sync.dma_start(
                x_bf[:, :].rearrange("(t p) d -> p t d", p=128), xb16[:, :]
            )
            nc.sync.dma_start(xT0[:, :], x_bf[:, 0:128], transpose=True)
            nc.sync.dma_start(xT1[:, :], x_bf[:, 128:256], transpose=True)

            # ---- y: cast-load chunks of 1024 rows, stage, transpose ----
            NCH = 8  # chunks
            RCH = M // NCH  # 1024 rows per chunk
            for c in range(NCH):
                y_src = y[c * RCH : (c + 1) * RCH, :].rearrange(
                    "(t p) d -> p t d", p=128
                )
                yf = spool.tile((128, (RCH // 128) * D), F32, name="yf", tag="yf")
                nc.sync.dma_start(yf[:, :], y_src)
                yb = spool.tile((128, (RCH // 128) * D), BF16, name="yb", tag="yb")
                nc.vector.tensor_copy(yb[:, :], yf[:, :])  # f32 -> bf16 cast
                nc.sync.dma_start(
                    y_bf[c * RCH : (c + 1) * RCH, :].rearrange(
                        "(t p) d -> p t d", p=128
                    ),
                    yb[:, :],
                )
            for c in range(NCH):
                nc.sync.dma_start(
                    yT0[:, c * RCH : (c + 1) * RCH],
                    y_bf[c * RCH : (c + 1) * RCH, 0:128],
                    transpose=True,
                )
                nc.sync.dma_start(
                    yT1[:, c * RCH : (c + 1) * RCH],
                    y_bf[c * RCH : (c + 1) * RCH, 128:256],
                    transpose=True,
                )

            # ---- y2 row: -0.5 * sum_d y[j,d]^2, as a [1, M] bf16 row ----
            # square yT slices on DVE, reduce over partitions with a
            # constant -0.5 column via the tensor engine.
            for t in range(M // 512):
                sl = slice(t * 512, (t + 1) * 512)
                sq0 = spool.tile((128, 512), BF16, name="sq0", tag="sq0")
                nc.vector.tensor_mul(sq0[:, :], yT0[:, sl], yT0[:, sl])
                sq1 = spool.tile((128, 512), BF16, name="sq1", tag="sq1")
                nc.vector.tensor_mul(sq1[:, :], yT1[:, sl], yT1[:, sl])
                psy2 = pspool.tile((1, 512), F32, name="psy2", tag="ps")
                nc.tensor.matmul(
                    psy2[:, :], neghalf_col[:, :], sq0[:, :], start=True, stop=False
                )
                nc.tensor.matmul(
                    psy2[:, :], neghalf_col[:, :], sq1[:, :], start=False, stop=True
                )
                nc.vector.tensor_copy(y2row[:, sl], psy2[:, :])

            # ---- main loop: 8 i-blocks x 4 j-supers of 2048 cols ----
            for b in range(XB):
                lhs0 = xT0[:, b * 128 : (b + 1) * 128]
                lhs1 = xT1[:, b * 128 : (b + 1) * 128]
                for js in range(M // 2048):
                    ps = pspool.tile((128, 2048), F32, name="ps", tag="ps")
                    # k-outer order: reuse each stationary operand across
                    # the 4 psum slices before switching weights
                    for jt in range(4):
                        sl = slice(js * 2048 + jt * 512, js * 2048 + (jt + 1) * 512)
                        nc.tensor.matmul(
                            ps[:, jt * 512 : (jt + 1) * 512],
                            lhs0,
                            yT0[:, sl],
                            start=True,
                            stop=False,
                        )
                    for jt in range(4):
                        sl = slice(js * 2048 + jt * 512, js * 2048 + (jt + 1) * 512)
                        nc.tensor.matmul(
                            ps[:, jt * 512 : (jt + 1) * 512],
                            lhs1,
                            yT1[:, sl],
                            start=False,
                            stop=False,
                        )
                    for jt in range(4):
                        sl = slice(js * 2048 + jt * 512, js * 2048 + (jt + 1) * 512)
                        nc.tensor.matmul(
                            ps[:, jt * 512 : (jt + 1) * 512],
                            ones_row[:, :],
                            y2row[:, sl],
                            start=False,
                            stop=True,
                        )
                    ob = opool.tile((128, 2048), F32, name="ob")
                    nc.scalar.activation(
                        ob[:, :],
                        ps[:, :],
                        AF.Exp,
                        bias=negx2[:, b : b + 1],
                        scale=2.0,
                    )
                    nc.sync.dma_start(
                        out[b * 128 : (b + 1) * 128, js * 2048 : (js + 1) * 2048],
                        ob[:, :],
                    )
    nc.finalize()
    return nc


def _get_nc() -> bass.Bass:
    if "nc" not in _NC_CACHE:
        _NC_CACHE["nc"] = _build_nc()
    return _NC_CACHE["nc"]


def kernel(x, y) -> np.ndarray:
    x = np.ascontiguousarray(np.asarray(x, dtype=np.float32))
    y = np.ascontiguousarray(np.asarray(y, dtype=np.float32))
    assert x.shape == (N, D) and y.shape == (M, D), (x.shape, y.shape)

    nc = _get_nc()
    in_maps = [
        {"x": x[c * NSH : (c + 1) * NSH], "y": y} for c in range(NCORES)
    ]
    res = run_bass_kernel_spmd(nc, in_maps, core_ids=list(range(NCORES)))
    return np.concatenate(
        [res.results[c]["out"] for c in range(NCORES)], axis=0
    )
